# revision 54
# baseline (speedup 1.0000x reference)
"""MoE top-2 routing kernel for 8 Trainium2 NeuronCores.

Strategy (expert parallelism per the sharding hint):
  Launch A (data-parallel gate): each core computes softmax gate + top-2
    combine weights for its 1024-token slice on device. The gate GEMM is
    computed transposed (token tile stationary, 8-wide W_g moving) over
    bf16 activations - half the DMA this kernel is bound by - and also
    reports each token's top2-top3 logit gap.
  Host: routing bookkeeping only - recomputes the exact fp32 gate for the
    ~400/8192 near-tie tokens the gap output flags (bf16 noise can only
    flip those), builds per-expert token index lists, pairs heavy experts
    with light ones (two cores per pair, each expert's tokens split across
    both), and gathers the pre-transposed bf16 token rows per core
    (measured end-to-end rel err 2.6e-3 vs the 2e-2 budget).
  Launch B (expert-parallel, load-balanced): each core runs the grouped
    GEMM for its two resident experts with tokens on the matmul FREE dim
    (weights stationary, psum [o_chunk, token]), so PE cost scales with
    the exact 2117-token capacity instead of a 128-quantized 2176 - no
    gathers, no transposes, no idle PE. The first 512-token slab is
    emitted k-major so matmuls start as soon as the k-th weight tile
    lands; expert B's weight stream and the last slab load are triggered
    mid-loop so bulk DMA can't head-of-line-block the tiny psum drains.
    Each drain is one fused DVE op ((psum + bias_col) * prob_rep -> fp16).
  Host: re-transposes each core's compact [O, cap] output and scatter-adds
    into the full [B, 2048] fp32 output (each token appears in exactly
    two experts' lists).
"""

import numpy as np
import ml_dtypes

import concourse.bass as bass
import concourse.mybir as mybir
from concourse.bass_utils import run_bass_kernel_spmd
from concourse.tile import TileContext

B = 8192
D = 2048
O = 2048
E = 8
P = 128
BS = B // E  # tokens per core in the gate launch
KT = D // P  # 16 k-tiles

f32 = mybir.dt.float32
bf16 = mybir.dt.bfloat16
f16 = mybir.dt.float16

BF16 = ml_dtypes.bfloat16


MAXW = 1  # this walrus build accepts one sync-wait command per instruction
_wsctr = [0]


def split_excess_waits(nc):
    """Post-pass: any instruction carrying more than MAXW sem-waits gets the
    excess moved onto spliced same-engine NoOps just before it (same-engine
    ge-waits executed earlier are semantically identical)."""
    import bass_rust

    for f in nc.m.functions:
        for blk in f.blocks:
            out = []
            changed = False
            for inst in blk.instructions:
                si = inst.sync_info
                if si is not None and len(si.on_wait) > MAXW:
                    waits = list(si.on_wait)
                    excess, keep = waits[:-MAXW], waits[-MAXW:]
                    for i in range(0, len(excess), MAXW):
                        _wsctr[0] += 1
                        nop = bass_rust.InstNoOp(
                            name=f"WSPLIT-{_wsctr[0]}", ins=[], outs=[]
                        )
                        nop.engine = inst.engine
                        nop.sync_info = mybir.SyncInfo(
                            on_wait=excess[i : i + MAXW], on_update=[]
                        )
                        out.append(nop)
                    inst.sync_info = mybir.SyncInfo(
                        on_wait=keep, on_update=list(si.on_update)
                    )
                    changed = True
                out.append(inst)
            if changed:
                blk.instructions = out


def build_gate_kernel():
    """Per core: gate for its BS-token slice. In: xt [D, E+BS] bf16 =
    W_g's 8 columns prepended to the core's xT slice, b_g [1, E] f32.
    Out: c [BS, E+1] f32 = top-2 masked softmax probs (zeros elsewhere)
    plus, in the last column, the top2-top3 logit gap.

    The gate GEMM is computed transposed (token tile stationary - ldweights
    is free on the PE - 8-wide W_g moving) so logits land in psum as
    [token, E], the exact layout the top-2/softmax chain needs. Activations
    stream in bf16, HALVING the DMA this kernel is bound by; bf16 logit
    noise (~3e-3) can flip only top-2 ties with gap < ~7e-3, so the host
    recomputes the exact fp32 gate for the ~400/8192 tokens whose reported
    gap is under 0.08 (10x margin) and trusts the device elsewhere. The
    softmax/top-k chain stays fp32 on the psum logits."""
    nc = bass.Bass()
    # xt carries W_g's 8 columns prepended to the 1024 tokens: the weight
    # load rides the first activation block instead of paying its own
    # 500ns descriptor slot in the DMA queue prefix
    xt = nc.dram_tensor("xt", [D, BS + E], bf16, kind="ExternalInput")
    bg = nc.dram_tensor("bg", [1, E], f32, kind="ExternalInput")
    cout = nc.dram_tensor("c", [BS, E + 1], f32, kind="ExternalOutput")
    NB = BS // 256  # DMA blocks (256 tokens: 512B runs keep the 1x rate)
    NG = BS // P

    with TileContext(nc) as tc:
        with (
            tc.tile_pool(name="const", bufs=1) as cpool,
            tc.tile_pool(name="work", bufs=2) as wpool,
            tc.tile_pool(name="psumg", bufs=4, space="PSUM") as pgpool,
        ):
            bgs = cpool.tile([1, E], f32)
            ones = cpool.tile([1, P], f32)
            nc.vector.memset(ones[:], 1.0)
            xts = cpool.tile([P, KT, BS + E], bf16)
            xt3 = xt.rearrange("(kt p) b -> p kt b", p=P)
            for bc in range(NB):
                a = 0 if bc == 0 else E + bc * 256
                b2 = E + (bc + 1) * 256
                nc.sync.dma_start(
                    out=xts[:, :, a:b2], in_=xt3[:, :, a:b2]
                )
                if bc == 0:
                    # 32-byte transfer but 500ns of descriptor time: keep it
                    # out of the activation stream's prefix (needed ~7us in)
                    nc.sync.dma_start(out=bgs[:], in_=bg[:, :])
            # gate bias replicated across partitions (bias is per-expert,
            # which is the free dim here)
            bg_ps = pgpool.tile([P, E], f32, tag="bg_ps")
            nc.tensor.matmul(
                bg_ps[:], lhsT=ones[:, :], rhs=bgs[:, :], start=True, stop=True
            )
            bg_rep = cpool.tile([P, E], f32)
            nc.vector.tensor_copy(bg_rep[:], bg_ps[:])
            # all groups' combine weights + tie gaps; one DMA at the end
            cc_all = cpool.tile([P, NG, E + 1], f32)

            for g in range(NG):
                g_ps = pgpool.tile([P, E], f32, tag="g_ps")
                for k in range(KT):
                    nc.tensor.matmul(
                        g_ps[:],
                        lhsT=xts[:, k, E + g * P : E + (g + 1) * P],
                        rhs=xts[:, k, 0:E],
                        start=(k == 0),
                        stop=(k == KT - 1),
                    )
                gl = wpool.tile([P, E], f32, tag="gl")
                nc.vector.tensor_add(gl[:], g_ps[:], bg_rep[:])
                # logits are bounded (|gl| < ~4 for this problem) so exp
                # can't overflow fp32: skip the max-subtraction and let the
                # Act exp run in parallel with the top-2 max8 on the DVE
                mx = wpool.tile([P, 8], f32, tag="mx")
                nc.vector.max(out=mx[:], in_=gl[:])
                ex = wpool.tile([P, E], f32, tag="ex")
                nc.scalar.activation(ex[:], gl[:], mybir.ActivationFunctionType.Exp)
                s = wpool.tile([P, 1], f32, tag="s")
                nc.vector.reduce_sum(out=s[:], in_=ex[:], axis=mybir.AxisListType.X)
                r = wpool.tile([P, 1], f32, tag="r")
                nc.vector.reciprocal(r[:], s[:])
                # top-2 mask on logits (order-preserving), applied to exps
                mskd = wpool.tile([P, E], f32, tag="mskd")
                nc.vector.scalar_tensor_tensor(
                    mskd[:],
                    in0=gl[:],
                    scalar=mx[:, 1:2],
                    in1=ex[:],
                    op0=mybir.AluOpType.is_ge,
                    op1=mybir.AluOpType.mult,
                )
                nc.vector.tensor_scalar_mul(cc_all[:, g, 0:E], mskd[:], r[:, 0:1])
                nc.vector.tensor_sub(
                    cc_all[:, g, E : E + 1], mx[:, 1:2], mx[:, 2:3]
                )
            nc.sync.dma_start(
                out=cout.rearrange("(t p) e -> p t e", p=P), in_=cc_all[:]
            )
    split_excess_waits(nc)
    return nc


def build_expert_kernel(capA=1097, capB=1020):
    """Per core: TWO experts (load balancing). Experts are paired
    heavy-with-light; each pair lives on two cores and each expert's tokens
    split across both. Tokens live on the matmul FREE dim (weights are the
    stationary lhsT, psum is [o_chunk, token]) so compute scales with the
    actual token capacity capA+capB=2117 instead of a 128-quantized 2176.
    Inputs arrive pre-gathered and pre-transposed (host routing):
    xg [KT, P, cap] bf16 (first capA tokens expert A, rest expert B),
    w [2*KT, P, O] bf16 both weights, bias as per-partition columns
    [P, 2*16], prob pre-replicated [P, cap]. Expert B's weight stream is
    triggered mid-loop where the DMA engines idle. The psum drain is a
    single fused DVE op: y16 = (psum + bias_col) * prob_rep, written back
    transposed [O, cap] in fp16 (host re-transposes)."""
    cap = capA + capB
    OC = O // P  # 16 o-chunks
    nc = bass.Bass()
    xg = nc.dram_tensor("xg", [KT, P, cap], bf16, kind="ExternalInput")
    w = nc.dram_tensor("w", [2 * KT, P, O], bf16, kind="ExternalInput")
    biasc = nc.dram_tensor("biasc", [P, 2 * OC], f32, kind="ExternalInput")
    probr = nc.dram_tensor("probr", [P, cap], f32, kind="ExternalInput")
    y = nc.dram_tensor("y", [O, cap], f16, kind="ExternalOutput")

    # token slabs within each expert region: lead with a 512-wide slab
    # (the k-major warmup wants max PE work per weight tile), then split
    # the remainder evenly - near-equal widths >=256 keep every bank's
    # drain well under its fill time (a 73-wide ragged slab measurably
    # outran the DVE drain + y16 pool and stalled the psum pipeline)
    slabs = []  # (offset, width, expert_half)
    for h, (base, csz) in enumerate([(0, capA), (capA, capB)]):
        widths = []
        rem = csz
        if h == 0:
            widths.append(min(512, rem))
            rem -= widths[0]
        np_parts = max(1, -(-rem // 512))
        for j in range(np_parts):
            lo = rem * j // np_parts
            hi = rem * (j + 1) // np_parts
            if hi > lo:
                widths.append(hi - lo)
        off = 0
        for wd in widths:
            slabs.append((base + off, wd, h))
            off += wd
    NSLAB = len(slabs)

    with TileContext(nc) as tc:
        with (
            tc.tile_pool(name="const", bufs=1) as cpool,
            tc.tile_pool(name="slab", bufs=3) as spool,
            tc.tile_pool(name="ps", bufs=2, space="PSUM") as ppool,
            tc.tile_pool(name="y16", bufs=6) as qpool,
        ):
            wk = [cpool.tile([P, O], bf16, name=f"wk{k}") for k in range(2 * KT)]
            xs = []

            def load_slab(s, ks=0, ke=KT):
                off, wd, _ = slabs[s]
                if ks == 0:
                    xs.append(spool.tile([P, KT, 512], bf16, tag="xs", name=f"xs{s}"))
                src = xg[ks:ke, :, off : off + wd].rearrange("k p c -> p k c")
                nc.sync.dma_start(out=xs[s][:, ks:ke, :wd], in_=src)

            ones = cpool.tile([1, P], bf16)
            nc.vector.memset(ones[:], 1.0)
            ones512 = cpool.tile([1, 512], bf16)
            nc.vector.memset(ones512[:], 1.0)
            # expert A weight stream is the warmup-critical path, and the
            # k-major warmup (o-chunks 0..7) only reads the FIRST HALF of
            # each weight tile's columns: stream half-tiles (0.73us arrival
            # granularity) interleaved with slab 0 so the first matmul
            # starts ~5.4us in; the second halves (first read by the
            # steady o8..15 phase ~33us in) follow
            load_slab(0, 0, KT // 4)
            nc.sync.dma_start(out=wk[0][:, : O // 2], in_=w[0, :, : O // 2])
            load_slab(0, KT // 4, KT // 2)
            for k in range(1, 9):
                nc.sync.dma_start(out=wk[k][:, : O // 2], in_=w[k, :, : O // 2])
            load_slab(0, KT // 2, KT)
            for k in range(9, KT):
                nc.sync.dma_start(out=wk[k][:, : O // 2], in_=w[k, :, : O // 2])
            # drain inputs: needed by the first psum drain (~33us), ahead
            # of the second-half weight stream
            bias_sb = cpool.tile([P, 2 * OC], f32)
            nc.sync.dma_start(out=bias_sb[:], in_=biasc[:, :])
            prob_rep = cpool.tile([P, cap], f32)
            nc.sync.dma_start(out=prob_rep[:, 0:512], in_=probr[:, 0:512])
            for k in range(KT):
                nc.sync.dma_start(out=wk[k][:, O // 2 :], in_=w[k, :, O // 2 :])
            nc.sync.dma_start(out=prob_rep[:, 512:], in_=probr[:, 512:])
            for s in range(1, min(4, NSLAB)):
                load_slab(s)

            # dummy matmuls on memset data: PE busy early so the p-state
            # ramp (3us of continuous busy) completes before the real GEMM
            for d in range(8):
                dmy = ppool.tile([P, 512], f32, tag=f"pb{d % 4}", name=f"dmy{d}")
                nc.tensor.matmul(
                    dmy[:], lhsT=ones[:, :], rhs=ones512[:, :], start=True,
                    stop=True,
                )

            def drain(pb, s, o):
                off, wd, h = slabs[s]
                col = h * OC + o
                # the very last bank is the kernel tail: split its drain so
                # the final DMA exposes half the latency
                parts = [0, wd // 2, wd] if s == NSLAB - 1 and o == OC - 1 else [0, wd]
                for j in range(len(parts) - 1):
                    a, b2 = parts[j], parts[j + 1]
                    y16 = qpool.tile([P, b2 - a], f16, tag=f"y16_{b2 - a}")
                    nc.vector.scalar_tensor_tensor(
                        y16[:],
                        in0=pb[:, a:b2],
                        scalar=bias_sb[:, col : col + 1],
                        in1=prob_rep[:, off + a : off + b2],
                        op0=mybir.AluOpType.add,
                        op1=mybir.AluOpType.mult,
                    )
                    nc.sync.dma_start(
                        out=y[o * P : (o + 1) * P, off + a : off + b2],
                        in_=y16[:],
                    )

            def wsel(h, k):
                return wk[h * KT + k]

            # slab 0, o-chunks 0..7: k-major over 8 psum banks so each
            # arriving weight tile unlocks 8 matmuls while the stream lands
            pb0 = [
                ppool.tile([P, 512], f32, tag=f"pb{o % 4}", name=f"pb0_{o}")
                for o in range(8)
            ]
            for k in range(KT):
                for o in range(8):
                    nc.tensor.matmul(
                        pb0[o][:],
                        lhsT=wk[k][:, o * P : (o + 1) * P],
                        rhs=xs[0][:, k, 0:512],
                        start=(k == 0),
                        stop=(k == KT - 1),
                    )
            for o in range(8):
                drain(pb0[o], 0, o)

            # steady state: one psum bank per (slab, o_chunk), k inner.
            # Expert B's weight stream is triggered from inside the loop so
            # its 25us of DMA can't queue ahead of the y-out drains.
            first = True
            for s in range(NSLAB):
                _, wd, h = slabs[s]
                for o in range(8 if first else 0, OC):
                    pb = ppool.tile([P, 512], f32, tag=f"pb{o % 4}")
                    for k in range(KT):
                        nc.tensor.matmul(
                            pb[:, :wd],
                            lhsT=wsel(h, k)[:, o * P : (o + 1) * P],
                            rhs=xs[s][:, k, :wd],
                            start=(k == 0),
                            stop=(k == KT - 1),
                        )
                    drain(pb, s, o)
                    if s == 0 and o % 2 == 1:
                        j = (o - 8) // 2
                        for k in range(KT + j * 4, KT + (j + 1) * 4):
                            nc.sync.dma_start(out=wk[k][:], in_=w[k, :, :])
                    # slab 4's buffer frees mid-drain-stream; trigger its
                    # load from here so the 5.8us transfer can't head-of-
                    # line-block the tiny y-out drains
                    if s == 3 and o == 3 and NSLAB > 4:
                        load_slab(4)
                first = False
    split_excess_waits(nc)
    return nc


_gate_nc = None
_exp_ncs = {}


def kernel(x, W_e, b_e, W_g, b_g):
    global _gate_nc
    x = np.ascontiguousarray(np.asarray(x, dtype=np.float32))
    W_e = np.ascontiguousarray(np.asarray(W_e, dtype=np.float32))
    b_e = np.ascontiguousarray(np.asarray(b_e, dtype=np.float32))
    W_g = np.ascontiguousarray(np.asarray(W_g, dtype=np.float32))
    b_g = np.ascontiguousarray(np.asarray(b_g, dtype=np.float32))

    xT = np.ascontiguousarray(x.T)  # [D, B] layout prep for the gate GEMM
    xTb = xT.astype(BF16)  # bf16 feeds BOTH launches (gate DMA halves)
    if _gate_nc is None:
        _gate_nc = build_gate_kernel()
    in_maps = [
        {
            "xt": np.ascontiguousarray(
                np.concatenate(
                    [W_g.astype(BF16), xTb[:, i * BS : (i + 1) * BS]], axis=1
                )
            ),
            "bg": b_g.reshape(1, E),
        }
        for i in range(E)
    ]
    res_a = run_bass_kernel_spmd(_gate_nc, in_maps, core_ids=list(range(8)))
    c9 = np.concatenate([r["c"] for r in res_a.results], axis=0)  # [B, E+1]
    c_full = np.ascontiguousarray(c9[:, :E])
    # Near-tie fix-up: bf16 logit noise (~3e-3) can only flip top-2
    # selections where the top2-top3 logit gap is < ~7e-3; recompute the
    # exact fp32 gate on host for tokens under a 10x-margin threshold
    # (~400 of 8192 tokens, a few M MACs of routing bookkeeping).
    risky = np.nonzero(c9[:, E] < 0.08)[0]
    if len(risky):
        lg = x[risky] @ W_g + b_g
        ex = np.exp(lg - lg.max(-1, keepdims=True))
        pr = ex / ex.sum(-1, keepdims=True)
        thr = np.sort(pr, -1)[:, -2:-1]
        c_full[risky] = np.where(pr >= thr, pr, 0.0)

    # Host routing bookkeeping: per-expert index lists from device-computed
    # c, then load balancing: pair the i-th heaviest expert with the i-th
    # lightest, give both weights to two cores, and split each expert's
    # tokens across them. Slot sizes fa/fb are chosen from the actual loads
    # (fa=9, fb=8 for this problem's inputs -> 17 chunks/core vs 18).
    sels = [np.nonzero(c_full[:, e] > 0.0)[0].astype(np.int32) for e in range(E)]
    n = np.array([len(s) for s in sels])
    order = np.argsort(-n, kind="stable")
    capA = (int(n[order[:4]].max()) + 1) // 2
    capB = (int(n[order[4:]].max()) + 1) // 2
    cap = capA + capB

    Wb = W_e.astype(BF16)

    if (capA, capB) not in _exp_ncs:
        _exp_ncs[(capA, capB)] = build_expert_kernel(capA, capB)
    OC = O // P
    in_maps = []
    scatter = []  # per core: list of (token_idx, col_offset)
    for i in range(4):
        p, q = int(order[i]), int(order[7 - i])
        hp = (n[p] + 1) // 2
        hq = (n[q] + 1) // 2
        wpair = np.ascontiguousarray(
            np.stack([Wb[p], Wb[q]]).reshape(2 * KT, P, O)
        )
        # per-partition bias columns: biasc[p, h*16+o] = b_h[o*128+p]
        biasc = np.ascontiguousarray(
            np.concatenate(
                [b_e[p].reshape(OC, P).T, b_e[q].reshape(OC, P).T], axis=1
            )
        )
        for half, (sp, sq) in enumerate(
            [(sels[p][:hp], sels[q][:hq]), (sels[p][hp:], sels[q][hq:])]
        ):
            assert len(sp) <= capA and len(sq) <= capB
            idxp = np.zeros(cap, np.int32)
            idxp[: len(sp)] = sp
            idxp[capA : capA + len(sq)] = sq
            probp = np.zeros(cap, np.float32)
            probp[: len(sp)] = c_full[sp, p]
            probp[capA : capA + len(sq)] = c_full[sq, q]
            in_maps.append(
                {
                    "xg": np.ascontiguousarray(xTb[:, idxp]).reshape(KT, P, cap),
                    "w": wpair,
                    "biasc": biasc,
                    "probr": np.ascontiguousarray(
                        np.broadcast_to(probp[None, :], (P, cap))
                    ),
                }
            )
            scatter.append([(sp, 0), (sq, capA)])
    res_b = run_bass_kernel_spmd(
        _exp_ncs[(capA, capB)], in_maps, core_ids=list(range(8))
    )

    out = np.zeros((B, O), np.float32)
    for core in range(8):
        yT = np.ascontiguousarray(res_b.results[core]["y"].T)  # [cap, O]
        for sel, off in scatter[core]:
            out[sel] += yT[off : off + len(sel)]
    return out
